# revision 3
# baseline (speedup 1.0000x reference)
"""Pipelined GEMM kernel for Trainium2, 8 NeuronCores.

Computes C = A @ B + ws*(ws+1)/2 with A:(8192,256) B:(256,8192) fp32.

Sharding: 2x4 grid over (M, N). Core (mi, ni) computes the
(4096, 2048) output block C[mi] x [ni]. No inter-core communication.

I/O precision: A and B are cast to bf16 on the host (the PE consumes
bf16 anyway), and C is stored to HBM as bf16 and upcast to fp32 on the
host. That halves the kernel's HBM traffic vs fp32 I/O:
per-core 2 (A^T) + 1 (B) + 16 (C) = 19MB, vs 38MB. At ~358 GB/s
HBM-per-NC this is a ~53us DMA floor; bf16 PE time for the
(4096x2048x256) block is ~55us warm, so the two overlap.
Output rounding to bf16 costs ~1e-3 norm rel err on top of the ~1e-3
from bf16 inputs (gate is 2e-2).

Per-core kernel (Tile framework):
  - A^T shard staged K-major (contraction dim on partitions), loaded as
    bf16 directly (no on-device cast); B likewise. Pieces ordered so the
    first m-tiles' operands arrive first.
  - Main loop over 32 m-tiles: 2(k) x 4(n) bf16 matmuls accumulate into
    [128, 1024] fp32 PSUM tiles (2 banks); +const is fused into the
    PSUM->SBUF copyback (alternating DVE / ACT) which also casts to
    bf16; two m-tiles share one 1MB store DMA, alternating between the
    two HWDGE rings (sync / scalar), with the last group split into
    0.25MB pieces to shorten the kernel's serial tail.
"""

import contextlib

import ml_dtypes
import numpy as np

import concourse.mybir as mybir
import concourse.tile as tile
from concourse import bacc
from concourse.bass_utils import run_bass_kernel_spmd

M, K, N = 8192, 256, 8192
NCORES = 8
RM, RN = 2, 4  # core grid over (M, N)
MS = M // RM  # 4096 rows of C per core
NS = N // RN  # 2048 cols of C per core
P = 128
MT = MS // P  # 32 m-tiles
KT = K // P  # 2 k-tiles
NCHUNK = 512  # one fp32 PSUM bank / max matmul free dim
NT = NS // NCHUNK  # 4 n-chunks = one [128, 2048] output tile per m-tile

F32 = mybir.dt.float32
BF16 = mybir.dt.bfloat16


def build_program(const_add: float, repeat: int = 1, loop_opts: dict | None = None,
                  tail_split: bool = True, psum_bufs: int = 4, opool_bufs: int = 4,
                  timing: bool = False):
    """repeat>1 wraps the whole body in a HW loop - used only by the
    timing harness. timing=True additionally makes `c` an Internal DRAM
    scratch tensor and adds a tiny dummy ExternalOutput, so a timing
    execution doesn't ship 32MB/core of outputs over the axon tunnel
    (the kernel's DMA work is unchanged)."""
    nc = bacc.Bacc("TRN2", target_bir_lowering=False, debug=False)
    at = nc.dram_tensor("at", [K, MS], BF16, kind="ExternalInput")
    b = nc.dram_tensor("b", [K, NS], BF16, kind="ExternalInput")
    c_kind = "Internal" if timing else "ExternalOutput"
    c = nc.dram_tensor("c", [MS, NS], BF16, kind=c_kind)
    dummy = (nc.dram_tensor("tout", [P, 16], BF16, kind="ExternalOutput")
             if timing else None)

    with tile.TileContext(nc) as tc:
        with (
            tc.tile_pool(name="bpool", bufs=1) as bpool,
            tc.tile_pool(name="atpool", bufs=1) as atpool,
            tc.tile_pool(name="psum", bufs=psum_bufs, space="PSUM") as psum_pool,
            tc.tile_pool(name="opool", bufs=opool_bufs) as opool,
            tc.For_i(0, repeat, 1, **(loop_opts or {}))
            if repeat > 1 else contextlib.nullcontext(),
        ):
            at_sb = [
                atpool.tile([P, MS], BF16, name=f"at{k}", tag=f"at{k}")
                for k in range(KT)
            ]
            b_sb = [
                bpool.tile([P, NS], BF16, name=f"b{k}", tag=f"b{k}")
                for k in range(KT)
            ]

            # Direct bf16 loads, ordered so the first m-tiles can start
            # immediately: a small head of A^T, all of B, then the rest
            # of A^T. Alternate the two HWDGE rings.
            AHEAD = 512  # first 4 m-tiles' worth of A^T columns
            pieces = []
            for k in range(KT):
                pieces.append((at_sb[k], at[k * P : (k + 1) * P, :], 0, AHEAD))
            for k in range(KT):
                pieces.append((b_sb[k], b[k * P : (k + 1) * P, :], 0, NS))
            for k in range(KT):
                pieces.append((at_sb[k], at[k * P : (k + 1) * P, :], AHEAD,
                               (MS - AHEAD) // 2))
            for k in range(KT):
                pieces.append((at_sb[k], at[k * P : (k + 1) * P, :],
                               AHEAD + (MS - AHEAD) // 2,
                               MS - AHEAD - (MS - AHEAD) // 2))
            for i, (dst, src, col0, width) in enumerate(pieces):
                eng = nc.sync if i % 2 == 0 else nc.scalar
                eng.dma_start(dst[:, col0 : col0 + width],
                              src[:, col0 : col0 + width])

            # Main GEMM loop; two m-tiles share one output tile so each
            # store DMA moves 1MB of bf16.
            for m2 in range(MT // 2):
                ot = opool.tile([P, 2 * NS], BF16)
                for mh in range(2):
                    m = m2 * 2 + mh
                    for jj in range(NT // 2):
                        ps = psum_pool.tile([P, 2 * NCHUNK], F32)
                        for j2 in range(2):
                            jc = jj * 2 + j2
                            for k in range(KT):
                                nc.tensor.matmul(
                                    ps[:, j2 * NCHUNK : (j2 + 1) * NCHUNK],
                                    at_sb[k][:, m * P : (m + 1) * P],
                                    b_sb[k][:, jc * NCHUNK : (jc + 1) * NCHUNK],
                                    start=(k == 0),
                                    stop=(k == KT - 1),
                                )
                        # +const fused into PSUM->SBUF eviction (casts to
                        # bf16 on write)
                        dst = ot[:, mh * NS + jj * 2 * NCHUNK
                                 : mh * NS + (jj + 1) * 2 * NCHUNK]
                        if (m + jj) % 2 == 0:
                            nc.vector.tensor_scalar_add(dst, ps[:], const_add)
                        else:
                            nc.scalar.activation(
                                dst, ps[:],
                                mybir.ActivationFunctionType.Copy,
                                bias=const_add,
                            )
                # stores alternate between the two HWDGE rings; the last
                # group is split into 0.25MB pieces on both rings so the
                # kernel's serial tail (final copyback + store drain) is
                # as short as possible.
                if m2 < MT // 2 - 1 or not tail_split:
                    dma_eng = nc.sync if m2 % 2 == 0 else nc.scalar
                    dst_ap = c[m2 * 2 * P : (m2 + 1) * 2 * P, :].rearrange(
                        "(h p) n -> p h n", p=P
                    )
                    dma_eng.dma_start(dst_ap, ot[:])
                else:
                    for mh in range(2):
                        m = m2 * 2 + mh
                        for nh in range(2):
                            dma_eng = nc.sync if nh % 2 == 0 else nc.scalar
                            dma_eng.dma_start(
                                c[m * P : (m + 1) * P,
                                  nh * (NS // 2) : (nh + 1) * (NS // 2)],
                                ot[:, mh * NS + nh * (NS // 2)
                                   : mh * NS + (nh + 1) * (NS // 2)],
                            )
            if dummy is not None:
                nc.sync.dma_start(dummy[:], b_sb[0][:, :16])

    nc.compile()
    return nc


_CACHE = {}


def _get_program(const_add: float):
    key = const_add
    if key not in _CACHE:
        _CACHE[key] = build_program(const_add)
    return _CACHE[key]


def make_in_maps(A, B):
    """2x4 (M, N) grid; A shards staged K-major; both inputs cast to
    bf16 on the host (the PE consumes bf16 regardless)."""
    A16 = np.asarray(A, dtype=ml_dtypes.bfloat16)
    B16 = np.asarray(B, dtype=ml_dtypes.bfloat16)
    maps = []
    for i in range(NCORES):
        mi, ni = divmod(i, RN)
        maps.append({
            "at": np.ascontiguousarray(A16[mi * MS : (mi + 1) * MS].T),
            "b": np.ascontiguousarray(B16[:, ni * NS : (ni + 1) * NS]),
        })
    return maps


def assemble(results):
    rows = []
    for mi in range(RM):
        rows.append(np.concatenate(
            [np.asarray(results[mi * RN + ni]["c"], dtype=np.float32)
             for ni in range(RN)], axis=1))
    return np.concatenate(rows, axis=0)


def run(A, B, world_size, trace=False, **spmd_kwargs):
    A = np.ascontiguousarray(np.asarray(A, dtype=np.float32))
    B = np.ascontiguousarray(np.asarray(B, dtype=np.float32))
    ws = int(world_size)
    const_add = float(ws * (ws + 1) / 2)
    assert A.shape == (M, K) and B.shape == (K, N)

    nc = _get_program(const_add)
    res = run_bass_kernel_spmd(
        nc, make_in_maps(A, B), list(range(NCORES)), trace=trace, **spmd_kwargs
    )
    return assemble(res.results), res


def kernel(A, B, world_size, **_unused):
    out, _ = run(A, B, world_size, trace=False)
    return out
